# revision 5
# baseline (speedup 1.0000x reference)
"""CentroidInstanceLoss on 8 Trainium2 NeuronCores (Bass/Tile), v3.

Subbatch-parallel (core c = subbatch c, points label-sorted on host, padded
to T*128; no collectives). Host precomputes one-hots and per-point pull
weights from the integer inputs. Device work per 128-point tile:
  pass1: Act square (batched x8), DVE reduce -> ss (x8), DVE scale xn=-x/|x|,
         PE matmul one-hot -> segment sums.
  pass2: PE gather matmul mu_pt then identity-matmul adds xn, so PSUM holds
         mu_pt - xhat; one batched DVE abs-reduce (x4) -> L1 distances.
  push:  rotation matmul + neg-identity matmul -> mu_rot - mu in PSUM,
         DVE abs-reduce per rotation (32 rotations; symmetry fills the rest).
Normalized points stay resident in SBUF; x is read from HBM once.
"""

import numpy as np

import concourse.bass as bass
import concourse.bacc as bacc
import concourse.mybir as mybir
import concourse.tile as tile

f32 = mybir.dt.float32
f16 = mybir.dt.float16

N_TOTAL = 262144
D = 256
S = 8
L = 64
NCORES = 8
DELTA_V = 0.5
DELTA_D = 1.5
NROT = L // 2

AluOp = mybir.AluOpType
ActFn = mybir.ActivationFunctionType
Axis = mybir.AxisListType


def build_nc(T: int, reps: int = 1, phases: tuple = ("p1", "p2", "push")):
    SG = 32  # norm super-group (tiles)
    G = 8    # DMA / batch group (tiles)
    PB = 4   # pass-2 psum batch (tiles per abs-reduce)
    assert T % SG == 0 and SG % G == 0 and G % PB == 0

    nc = bacc.Bacc("TRN2", target_bir_lowering=False, debug=False, num_devices=1)

    x_in = nc.dram_tensor("x", [128, T * D], f16, kind="ExternalInput")
    oh_in = nc.dram_tensor("oh", [128, T * L], f16, kind="ExternalInput")
    ohT_in = nc.dram_tensor("oht", [L, T * 128], f16, kind="ExternalInput")
    wpt_in = nc.dram_tensor("wpt", [128, T], f32, kind="ExternalInput")
    crecipn_in = nc.dram_tensor("crecipn", [L, 1], f32, kind="ExternalInput")
    perms_in = nc.dram_tensor("perms", [L, NROT * L], f16, kind="ExternalInput")
    negident_in = nc.dram_tensor("negident", [L, L], f16, kind="ExternalInput")
    ident_in = nc.dram_tensor("ident", [128, 128], f16, kind="ExternalInput")

    lpull_out = nc.dram_tensor("lpull", [128, 1], f32, kind="ExternalOutput")
    qrot_out = nc.dram_tensor("qrot", [L, NROT + 1], f32, kind="ExternalOutput")

    with tile.TileContext(nc) as tc:
        with (
            tc.tile_pool(name="const", bufs=1) as constp,
            tc.tile_pool(name="big", bufs=1) as bigp,
            tc.tile_pool(name="xb", bufs=4) as xbp,
            tc.tile_pool(name="ohb", bufs=4) as ohbp,
            tc.tile_pool(name="sink", bufs=2) as sinkp,
            tc.tile_pool(name="small", bufs=3) as smallp,
            tc.tile_pool(name="mu", bufs=1) as mup,
        ):
            wpt_sb = constp.tile([128, T], f32)
            nc.sync.dma_start(wpt_sb[:], wpt_in[:])
            crecipn_sb = constp.tile([L, 1], f32)
            nc.sync.dma_start(crecipn_sb[:], crecipn_in[:])
            perms_sb = constp.tile([L, NROT * L], f16)
            nc.sync.dma_start(perms_sb[:], perms_in[:])
            negident_sb = constp.tile([L, L], f16)
            nc.sync.dma_start(negident_sb[:], negident_in[:])
            ident_sb = constp.tile([128, 128], f16)
            nc.sync.dma_start(ident_sb[:], ident_in[:])
            negdv_sb = constp.tile([128, 1], f32)
            nc.vector.memset(negdv_sb[:], -DELTA_V)

            for rep in range(reps):
                ss_all = bigp.tile([128, T], f32, tag="ss", name="ss_all")
                rr_all = bigp.tile([128, T], f32, tag="rr", name="rr_all")
                xn_all = bigp.tile([128, T, D], f16, tag="xn", name="xn_all")
                d1_all = bigp.tile([128, T], f32, tag="d1", name="d1_all")
                mu16 = mup.tile([L, D], f16, tag="mu16", name="mu16")

                # ---- pass 1 ----
                with tc.tile_pool(name="p1ps", bufs=1, space="PSUM") as p1ps:
                    ps_mu = p1ps.tile([L, D], f32, tag="mu", name="ps_mu")
                    if "p1" not in phases:
                        nc.vector.memset(ss_all[:], 1.0)
                        nc.vector.memset(rr_all[:], 1.0)
                        nc.vector.memset(xn_all[:, 0, :], 0.0)
                        nc.vector.memset(ps_mu[:], 0.0)
                    for sg in range(T // SG if "p1" in phases else 0):
                        xbs, ohbs = [], []
                        for g in range(SG // G):
                            t0 = sg * SG + g * G
                            xb = xbp.tile([128, G, D], f16, tag="xb")
                            nc.sync.dma_start(
                                xb[:], x_in[:, t0 * D:(t0 + G) * D].rearrange(
                                    "p (g d) -> p g d", g=G)
                            )
                            ohb = ohbp.tile([128, G, L], f16, tag="ohb")
                            nc.sync.dma_start(
                                ohb[:], oh_in[:, t0 * L:(t0 + G) * L].rearrange(
                                    "p (g l) -> p g l", g=G)
                            )
                            xbs.append(xb)
                            ohbs.append(ohb)
                            sqk = sinkp.tile([128, G, D], f16, tag="sqk")
                            nc.scalar.activation(
                                sqk[:], xb[:], ActFn.Square
                            )
                            nc.vector.tensor_reduce(
                                ss_all[:, t0:t0 + G], sqk[:],
                                axis=Axis.X, op=AluOp.add,
                            )
                        sgsl = slice(sg * SG, (sg + 1) * SG)
                        sq = smallp.tile([128, SG], f32, tag="sq")
                        nc.scalar.activation(sq[:], ss_all[:, sgsl], ActFn.Sqrt)
                        nc.vector.tensor_scalar_add(sq[:], sq[:], 1e-8)
                        nc.vector.reciprocal(rr_all[:, sgsl], sq[:])
                        for g in range(SG // G):
                            for j in range(G):
                                t = sg * SG + g * G + j
                                xnt = xn_all[:, t, :]
                                nc.vector.tensor_scalar(
                                    xnt, xbs[g][:, j, :],
                                    rr_all[:, t:t + 1], -1.0,
                                    op0=AluOp.mult, op1=AluOp.mult,
                                )
                                nc.tensor.matmul(
                                    ps_mu[:], ohbs[g][:, j, :], xnt,
                                    start=(t == 0), stop=(t == T - 1),
                                )
                    # mu = (-sums) * (-1/count)
                    nc.vector.tensor_scalar(
                        mu16[:], ps_mu[:], crecipn_sb[:, 0:1], None,
                        op0=AluOp.mult,
                    )

                # ---- pass 2: pull distances ----
                with (
                    tc.tile_pool(name="ohtb", bufs=3) as ohtp,
                    tc.tile_pool(name="p2ps", bufs=3, space="PSUM") as p2ps,
                ):
                    for g in range(T // G if "p2" in phases else 0):
                        ohtb = ohtp.tile([L, G, 128], f16, tag="ohtb")
                        nc.sync.dma_start(
                            ohtb[:],
                            ohT_in[:, g * G * 128:(g + 1) * G * 128].rearrange(
                                "l (g p) -> l g p", g=G)
                        )
                        for q in range(G // PB):
                            ps4 = p2ps.tile([128, PB, D], f32, tag="pt")
                            for j in range(PB):
                                t = g * G + q * PB + j
                                nc.tensor.matmul(
                                    ps4[:, j, :],
                                    ohtb[:, q * PB + j, :], mu16[:],
                                    start=True, stop=False,
                                )
                                nc.tensor.matmul(
                                    ps4[:, j, :],
                                    ident_sb[:], xn_all[:, t, :],
                                    start=False, stop=True,
                                )
                            tq = g * G + q * PB
                            nc.vector.tensor_reduce(
                                d1_all[:, tq:tq + PB], ps4[:],
                                axis=Axis.X, op=AluOp.add,
                                apply_absolute_value=True,
                            )
                    if "p2" in phases:
                        t1 = bigp.tile([128, T], f32, tag="t1", name="t1")
                        nc.scalar.activation(
                            t1[:], d1_all[:], ActFn.Relu, bias=negdv_sb[:]
                        )
                        sink3 = bigp.tile([128, T], f32, tag="s3", name="sink3")
                        lp = smallp.tile([128, 1], f32, tag="lp")
                        t2 = bigp.tile([128, T], f32, tag="t2", name="t2")
                        nc.vector.tensor_mul(t2[:], t1[:], t1[:])
                        nc.vector.scalar_tensor_tensor(
                            sink3[:], t2[:], 1.0, wpt_sb[:],
                            op0=AluOp.bypass, op1=AluOp.mult,
                            accum_out=lp[:],
                        )
                    else:
                        lp = smallp.tile([128, 1], f32, tag="lp")
                        nc.vector.memset(lp[:], 0.0)
                    nc.sync.dma_start(lpull_out[:], lp[:])

                # ---- push ----
                if "push" in phases:
                    q_sb = mup.tile([L, NROT + 1], f32, tag="q", name="q_sb")
                    nc.vector.memset(q_sb[:, 0:1], 0.0)
                    with tc.tile_pool(name="rotps", bufs=2, space="PSUM") as rotpsp:
                        for k in range(1, NROT + 1):
                            ps_rot = rotpsp.tile([L, D], f32, tag="rotps")
                            nc.tensor.matmul(
                                ps_rot[:],
                                perms_sb[:, (k - 1) * L:k * L], mu16[:],
                                start=True, stop=False,
                            )
                            nc.tensor.matmul(
                                ps_rot[:], negident_sb[:], mu16[:],
                                start=False, stop=True,
                            )
                            nc.vector.tensor_reduce(
                                q_sb[:, k:k + 1], ps_rot[:],
                                axis=Axis.X, op=AluOp.add,
                                apply_absolute_value=True,
                            )
                    nc.sync.dma_start(qrot_out[:], q_sb[:])
                else:
                    zq = smallp.tile([L, NROT + 1], f32, tag="zq")
                    nc.vector.memset(zq[:], 0.0)
                    nc.sync.dma_start(qrot_out[:], zq[:])

    nc.compile()
    return nc


def make_in_maps(outputs: np.ndarray, labels: np.ndarray, subbatch: np.ndarray):
    n = outputs.shape[0]
    labels = np.asarray(labels, dtype=np.int64)
    subbatch = np.asarray(subbatch, dtype=np.int64)

    per_core = []
    max_nc = 0
    for c in range(NCORES):
        idx = np.flatnonzero(subbatch == c)
        lab = labels[idx]
        order = np.argsort(lab, kind="stable")
        idx_sorted = idx[order]
        labs = lab[order]
        counts = np.bincount(labs, minlength=L).astype(np.float64)
        per_core.append((idx_sorted, labs, counts))
        max_nc = max(max_nc, idx.size)

    n_pad = max(36864, -(-max_nc // 4096) * 4096)
    T = n_pad // 128

    pp, kk, aa = np.meshgrid(
        np.arange(L), np.arange(1, NROT + 1), np.arange(L), indexing="ij"
    )
    perms = (pp == (aa + kk) % L).astype(np.float16)  # [L, NROT, L]
    perms = np.ascontiguousarray(perms.reshape(L, NROT * L))
    negident = (-np.eye(L)).astype(np.float16)
    ident = np.eye(128).astype(np.float16)

    in_maps = []
    tables = []
    for c in range(NCORES):
        idx_sorted, labs, counts = per_core[c]
        n_c = idx_sorted.size
        M = float((counts > 0).sum())
        valid = M > 1.0

        xarr = np.zeros((n_pad, D), dtype=np.float16)
        xarr[:n_c] = outputs[idx_sorted]
        x_l = np.ascontiguousarray(
            xarr.reshape(T, 128, D).transpose(1, 0, 2).reshape(128, T * D)
        )

        ohfull = np.zeros((n_pad, L), dtype=np.float16)
        ohfull[np.arange(n_c), labs] = 1.0
        oh_l = np.ascontiguousarray(
            ohfull.reshape(T, 128, L).transpose(1, 0, 2).reshape(128, T * L)
        )

        ohT = np.zeros((L, n_pad), dtype=np.float16)
        ohT[labs, np.arange(n_c)] = 1.0

        w = (
            np.where(counts > 0, 1.0 / (M * np.maximum(counts, 1.0)), 0.0)
            if valid else np.zeros(L)
        ).astype(np.float32)
        wfull = np.zeros((n_pad,), dtype=np.float32)
        wfull[:n_c] = w[labs]
        wpt = np.ascontiguousarray(wfull.reshape(T, 128).T)

        crecipn = (-1.0 / np.maximum(counts, 1.0)).astype(np.float32)

        in_maps.append({
            "x": x_l,
            "oh": oh_l,
            "oht": ohT,
            "wpt": wpt,
            "crecipn": crecipn.reshape(L, 1),
            "perms": perms,
            "negident": negident,
            "ident": ident,
        })
        tables.append((counts, M, valid))
    return in_maps, (tables, T)


def combine(results, tables_T, n: int):
    tables, T = tables_T
    pull_total = np.float64(0.0)
    push_total = np.float64(0.0)
    a = np.arange(L)
    for c in range(NCORES):
        counts, M, valid = tables[c]
        if not valid:
            continue
        pull_total += results[c]["lpull"].astype(np.float64).sum()
        q = results[c]["qrot"].astype(np.float64)  # [L, NROT+1]
        dist = np.zeros((L, L))
        for k in range(1, NROT + 1):
            dist[a, (a + k) % L] = q[:, k]
        for k in range(NROT + 1, L):
            dist[a, (a + k) % L] = dist[(a + k) % L, a]
        present = counts > 0
        mask = present[:, None] & present[None, :] & ~np.eye(L, dtype=bool)
        r = np.maximum(2.0 * DELTA_D - dist, 0.0) ** 2
        push_total += np.where(mask, r, 0.0).sum() / max(M * (M - 1.0), 1.0)
    return np.float32((pull_total + push_total) / n)


_NC_CACHE: dict = {}


def _get_nc(T: int):
    if T not in _NC_CACHE:
        _NC_CACHE[T] = build_nc(T)
    return _NC_CACHE[T]


def kernel(outputs, labels, subbatch_indices):
    from concourse.bass_utils import run_bass_kernel_spmd

    outputs = np.asarray(outputs, dtype=np.float32)
    labels = np.asarray(labels, dtype=np.int32)
    subbatch_indices = np.asarray(subbatch_indices, dtype=np.int32)

    in_maps, tables_T = make_in_maps(outputs, labels, subbatch_indices)
    nc = _get_nc(tables_T[1])
    res = run_bass_kernel_spmd(nc, in_maps, list(range(NCORES)))
    return combine(res.results, tables_T, outputs.shape[0])
